# revision 75
# baseline (speedup 1.0000x reference)
"""Multi-head attention (B=4, S=2048, D=1024, H=16) on 8 trn2 NeuronCores.

Sharding: 2 cores per batch element, split by HEADS (tensor parallel): core
(b, half) owns heads [8*half, 8*half+8) of batch b for the full S=2048
query rows.  Q/K/V projections use host-sliced weight columns (512 dims per
core), so no projection work is duplicated; the O projection produces a
partial product out_part = X_local @ Wo[:, local].T which the host sums
across the two cores of a batch (and adds bo).

Host-side layout prep (part of sharding): inputs arrive pre-transposed and
pre-cast to bf16 — xT [d, tokens], mask.T [s, r] as bf16 {0,1}, weights
W.T [d_in, d_out_slice].  This removes every PE transpose / GPSIMD cast of
the old pipeline; contraction operands stream straight from DRAM.

Per-core pipeline (all intermediates SBUF-resident, no DRAM scratch):
  1. Qt[o,r] / Kt[o,s] (o = 128 dims per head pair, bias via
     tensor_scalar_add) and V[s,d] (+bv broadcast) in bf16, N=512 matmuls.
  2. Attention per pair p (2 heads packed), per rc (512 query cols):
       sc[s_tile, 1024] psum = h0|h1 scores (tile_position row packing,
       K=64 concurrent halves)
       pexp = exp(0.125*sc) bf16  (ONE activation per (rc, st) — ACT is
       the critical engine; scores bounded => no max pass)
       pexp *= maskT slice (DVE, 2x bf16, per head half)
       xt[65, 512] += [V_h|1].T @ pexp_h  (psum row 64 = softmax denom)
     normalize: evict xt -> SBUF, reciprocal of denom row, partition-
     broadcast on GPSIMD (deferred past the rc boundary), DVE multiply
     -> Xt bf16.
  3. out_part[r, o] = Xt.T @ WoT (bf16, accumulated over local d), bf16
     out partials (the host sums the two head-halves + bo in f32).
  Projections for pair p+1 and the O projection are interleaved into the
  attention loop's PE slack (program-order paced per r-chunk) so the
  ScalarE exp stream never starves and the PE never idles.

PSUM (8 banks): scores 2x[128,1024] (4) + xt/bcast 2x[65,512] (2) +
proj/V/O accum [128,512] x2 (2).
"""

import numpy as np

import concourse.bass as bass
import concourse.bacc as bacc
import concourse.mybir as mybir
import concourse.tile as tile

F32 = mybir.dt.float32
BF16 = mybir.dt.bfloat16

B, S, D, H, DK = 4, 2048, 1024, 16, 64
NCORES = 8
P = 128
DL = 512            # local head dims per core (8 heads)
NPAIR = 4           # local head pairs; pair p <-> o-tile p
ST = S // P         # 16 s-tiles
RC = 512            # r chunk (matmul free dim)
NRC = S // RC       # 4 r chunks (full 2048 query rows per core)
KT = D // P         # 8 contraction tiles for QKV proj
OKT = DL // P       # 4 contraction tiles for O proj


def build_nc():
    nc = bacc.Bacc("TRN2", target_bir_lowering=False, debug=False,
                   num_devices=NCORES)

    xqT = nc.declare_dram_parameter("xqT", [D, S], BF16, isOutput=False)
    xkT = nc.declare_dram_parameter("xkT", [D, S], BF16, isOutput=False)
    xvT = nc.declare_dram_parameter("xvT", [D, S], BF16, isOutput=False)
    mskT = nc.declare_dram_parameter("mskT", [S, S], BF16, isOutput=False)
    wqT = nc.declare_dram_parameter("wqT", [D, DL], BF16, isOutput=False)
    wkT = nc.declare_dram_parameter("wkT", [D, DL], BF16, isOutput=False)
    wvT = nc.declare_dram_parameter("wvT", [D, DL], BF16, isOutput=False)
    woT = nc.declare_dram_parameter("woT", [DL, D], BF16, isOutput=False)
    bq = nc.declare_dram_parameter("bq", [DL], F32, isOutput=False)
    bk = nc.declare_dram_parameter("bk", [DL], F32, isOutput=False)
    bv = nc.declare_dram_parameter("bv", [DL], F32, isOutput=False)
    # bf16 output partials: the host sums the two head-halves in f32;
    # halving the output wire shortens the endgame DMA drain that
    # pins the kernel end time.
    out = nc.declare_dram_parameter("out", [S, D], BF16, isOutput=True)

    xq_v = xqT.ap().rearrange("(t p) r -> p t r", p=P)
    xk_v = xkT.ap().rearrange("(t p) r -> p t r", p=P)
    xv_v = xvT.ap().rearrange("(t p) r -> p t r", p=P)
    mt_v = mskT.ap().rearrange("(t p) r -> p t r", p=P)
    wq_v = wqT.ap().rearrange("(t p) o -> p t o", p=P)
    wk_v = wkT.ap().rearrange("(t p) o -> p t o", p=P)
    wv_v = wvT.ap().rearrange("(t p) o -> p t o", p=P)
    wo_v = woT.ap().rearrange("(t p) o -> p t o", p=P)

    with tile.TileContext(nc) as tc:
        with (
            tc.tile_pool(name="const", bufs=1) as const,
            tc.tile_pool(name="persist", bufs=1) as persist,
            tc.tile_pool(name="wt", bufs=1) as wtp,
            tc.tile_pool(name="xc", bufs=3) as xcp,
            tc.tile_pool(name="xvc", bufs=3) as xvcp,
            tc.tile_pool(name="qk", bufs=2) as qkp,
            tc.tile_pool(name="oev", bufs=4) as oevp,
            tc.tile_pool(name="pexp", bufs=5) as pexpp,
            tc.tile_pool(name="norm", bufs=2) as normp,
            tc.tile_pool(name="ps_sc", bufs=2, space="PSUM") as scp,
            tc.tile_pool(name="ps_av", bufs=1, space="PSUM") as avp,
            tc.tile_pool(name="ps_pj", bufs=2, space="PSUM") as pjp,
        ):
            # bias tiles; DMAs issued in phase 0 (their ISSUE cost would
            # otherwise delay the startup-critical chunk stream)
            bq_sb = const.tile([P, NPAIR], F32)
            bk_sb = const.tile([P, NPAIR], F32)
            bv_bc = const.tile([P, DL], F32)

            def bias_dmas():
                nc.sync.dma_start(
                    out=bq_sb, in_=bq.ap().rearrange("(t p) -> p t", p=P))
                nc.sync.dma_start(
                    out=bk_sb, in_=bk.ap().rearrange("(t p) -> p t", p=P))
                bv_ap = bv.ap()
                nc.sync.dma_start(
                    out=bv_bc,
                    in_=bass.AP(tensor=bv_ap.tensor, offset=bv_ap.offset,
                                ap=[[0, P]] + bv_ap.ap.copy()))

            # mask.T resident [s_part, st, r] bf16; r-chunked, and issued
            # lazily (scalar DGE queue, idle until attention) so startup
            # bandwidth goes to the first projection chunks.
            mt_sb = persist.tile([P, ST, S], BF16)

            def mask_chunk(rc, half=None):
                # sync queue on purpose: program order on ONE queue is the
                # only wire-ordering tool (other queues' issue free-runs).
                ts = slice(0, ST) if half is None else \
                    slice(half * (ST // 2), (half + 1) * (ST // 2))
                nc.sync.dma_start(
                    out=mt_sb[:, ts, rc * RC:(rc + 1) * RC],
                    in_=mt_v[:, ts, rc * RC:(rc + 1) * RC])

            # weight tiles; DMAs are issued in phase 0 in exact need order
            # (the DMA backend drains transfers serially, so issue order IS
            # the schedule).  wo is DMA'd much later — only the O projection
            # needs it.
            wq_sb = wtp.tile([P, KT, DL], BF16, tag="wq")
            wk_sb = wtp.tile([P, KT, DL], BF16, tag="wk")
            wv_sb = wtp.tile([P, KT, DL], BF16, tag="wv")
            wo_sb = wtp.tile([P, OKT, D], BF16, tag="wo")

            # V (all pairs, +ones cols at 64/129) and Xt live for the whole
            # kernel.
            vext = persist.tile([P, NPAIR, ST, 130], BF16)
            for p in range(NPAIR):
                nc.vector.memset(vext[:, p, :, DK:DK + 1], 1.0)
                nc.vector.memset(vext[:, p, :, 129:130], 1.0)
            xt_sb = persist.tile([P, OKT, S], BF16)

            qt_tiles = {}

            def alloc_pair_tiles(p):
                qt = qkp.tile([P, S], BF16, tag="qt", name=f"qt{p}")
                kt = qkp.tile([P, S], BF16, tag="kt", name=f"kt{p}")
                qt_tiles[p] = (qt, kt)

            # ---------- interleavable work units ----------
            class QKProj:
                """Q or K projection for pair p; .load(rc)/.chain(rc) are
                separately orderable so the startup wire order can be tuned.
                units() gives the default prefetching sequence."""

                def __init__(self, p, which):
                    self.p = p
                    self.which = which
                    self.x_v = {"q": xq_v, "k": xk_v}[which]
                    self.w_sb = {"q": wq_sb, "k": wk_sb}[which]
                    self.b_sb = {"q": bq_sb, "k": bk_sb}[which]
                    self.chunks = {}

                def load(self, rc, split=False):
                    x_sb = xcp.tile([P, KT, RC], BF16, tag="xc",
                                    name=f"{self.which}{self.p}_ld{rc}")
                    src = self.x_v[:, :, rc * RC:(rc + 1) * RC]
                    if split:  # halve so the first chain starts sooner
                        h = KT // 2
                        nc.sync.dma_start(out=x_sb[:, :h, :],
                                          in_=src[:, :h, :])
                        nc.sync.dma_start(out=x_sb[:, h:, :],
                                          in_=src[:, h:, :])
                    else:
                        nc.sync.dma_start(out=x_sb, in_=src)
                    self.chunks[rc] = x_sb

                def chain(self, rc, prefetch=False):
                    if prefetch and rc + 1 < NRC:
                        self.load(rc + 1)
                    ps = pjp.tile([P, RC], F32, tag="pj", name="pj_ps")
                    for kt in range(KT):
                        nc.tensor.matmul(
                            ps, self.w_sb[:, kt, self.p * P:(self.p + 1) * P],
                            self.chunks[rc][:, kt, :],
                            start=(kt == 0), stop=(kt == KT - 1))
                    dst = qt_tiles[self.p][0 if self.which == "q" else 1]
                    nc.vector.tensor_scalar_add(
                        out=dst[:, rc * RC:(rc + 1) * RC], in0=ps,
                        scalar1=self.b_sb[:, self.p:self.p + 1])

                def units(self):
                    return ([lambda: self.load(0)] +
                            [lambda rc=rc: self.chain(rc, prefetch=True)
                             for rc in range(NRC)])

            xv_chunks = {}

            def xv_load(sv):
                x_sb = xvcp.tile([P, KT, P], BF16, tag="xv", name=f"xv_ld{sv}")
                nc.sync.dma_start(out=x_sb,
                                  in_=xv_v[:, :, sv * P:(sv + 1) * P])
                xv_chunks[sv] = x_sb

            def v_unit(sv):
                """V proj for s-tile sv, all 512 local dims at once."""
                if sv + 3 < ST:
                    xv_load(sv + 3)
                ps = pjp.tile([P, DL], F32, tag="pj", name="vj_ps")
                for kt in range(KT):
                    nc.tensor.matmul(ps, xv_chunks[sv][:, kt, :],
                                     wv_sb[:, kt, :],
                                     start=(kt == 0), stop=(kt == KT - 1))
                # single fused bias-add writing the per-pair dual-head
                # layout: out ap iterates pair -> head -> dk, matching the
                # contiguous d order of the psum columns.
                dst = vext[:, 0, sv, :]
                nc.vector.tensor_add(
                    bass.AP(tensor=dst.tensor, offset=dst.offset,
                            ap=dst.ap[:1] +
                            [[ST * 130, NPAIR], [65, 2], [1, DK]]),
                    ps, bv_bc)

            def o_unit(mt_r, nn, alt=False):
                """out[mt_r*128.., nn*512..] partial (4 MM + DVE evict).
                alt=True borrows the (dead, post-attention) scores psum so
                the final chains double the rotation depth."""
                if alt:
                    ps = scp.tile([P, 2 * RC], F32, tag="sc",
                                  name="o_ps_alt")[:, 0:RC]
                else:
                    ps = pjp.tile([P, RC], F32, tag="pj", name="o_ps")
                for kt in range(OKT):
                    nc.tensor.matmul(
                        ps, xt_sb[:, kt, mt_r * P:(mt_r + 1) * P],
                        wo_sb[:, kt, nn * RC:(nn + 1) * RC],
                        start=(kt == 0), stop=(kt == OKT - 1))
                ev = oevp.tile([P, RC], BF16, tag="oev", name="o_ev")
                nc.vector.tensor_copy(out=ev, in_=ps)
                nc.sync.dma_start(
                    out=out[mt_r * P:(mt_r + 1) * P, nn * RC:(nn + 1) * RC],
                    in_=ev)

            # ---------- attention ----------
            def _emit_av(p, xt_ps, pexp, st):
                for h in range(2):
                    nc.tensor.matmul(
                        xt_ps[h], vext[:, p, st, h * 65:h * 65 + 65],
                        pexp[:, h * RC:(h + 1) * RC],
                        start=(st == 0), stop=(st == ST - 1))

            pending_norm = []

            def _normalize_front(p, rc, xt_ps):
                """DVE prefix at rc end: evict the AV accum (frees its psum
                banks) and compute 1/denom; the PE broadcast + final
                multiply are DEFERRED so the in-order PE queue never stalls
                on this DVE chain at an rc boundary."""
                for h in range(2):
                    xn = normp.tile([65, RC], F32, tag="xn", name="xn")
                    nc.vector.tensor_copy(out=xn, in_=xt_ps[h])
                    recip = normp.tile([1, RC], BF16, tag="recip",
                                       name="recip")
                    with nc.allow_low_precision(reason="bf16 denom recip"):
                        nc.vector.reciprocal(recip, xn[64:65, :])
                    pending_norm.append((p, rc, h, xn, recip))

            def flush_norm():
                """1/denom broadcast over partitions on the (idle) GPSIMD,
                then Xt = xn * bc."""
                for p, rc, h, xn, recip in pending_norm:
                    bc = normp.tile([DK, RC], BF16, tag=f"bc{h}", name="bc")
                    nc.gpsimd.partition_broadcast(bc, recip)
                    nc.vector.tensor_mul(
                        xt_sb[DK * h:DK * h + DK, p, rc * RC:(rc + 1) * RC],
                        xn[0:DK, :], bc)
                pending_norm.clear()

            def attn_pair(p, extras_by_rc, lead_rc0=False):
                """extras_by_rc: 4 lists of callables; list rc is emitted
                during r-chunk rc's st loop.  lead_rc0: emit rc0's units one
                per slot from the start (for V tiles racing the AV stream)
                instead of spreading them evenly."""
                qt, ktile = qt_tiles[p]
                for rc in range(NRC):
                    extra = [flush_norm] + extras_by_rc[rc]
                    lead = lead_rc0 and rc == 0
                    ei = 0
                    pend = None
                    xt_ps = [avp.tile([65, RC], F32, tag=f"xt{h}",
                                      name=f"xt_ps{h}") for h in range(2)]
                    for st in range(ST):
                        sc = scp.tile([P, 2 * RC], F32, tag="sc",
                                      name="sc_ps")
                        for h in range(2):
                            hp = h * DK
                            nc.tensor.matmul(
                                sc[:, h * RC:(h + 1) * RC],
                                ktile[hp:hp + DK, st * P:(st + 1) * P],
                                qt[hp:hp + DK, rc * RC:(rc + 1) * RC],
                                start=True, stop=True,
                                tile_position=(hp, 0))
                        pexp = pexpp.tile([P, 2 * RC], BF16, tag="pexp",
                                          name="pexp")
                        nc.scalar.activation(
                            pexp, sc, mybir.ActivationFunctionType.Exp,
                            scale=0.125)
                        msl = mt_sb[:, st, rc * RC:(rc + 1) * RC]
                        nc.vector.tensor_mul(
                            pexp, pexp,
                            bass.AP(tensor=msl.tensor, offset=msl.offset,
                                    ap=msl.ap[:1] + [[0, 2]] + msl.ap[1:]))
                        if pend is not None:
                            _emit_av(p, xt_ps, *pend)
                        pend = (pexp, st)
                        slot = st + 1
                        # pace over ST+1 so ~1 unit drains AFTER the last
                        # AV — giving the PE work while ACT catches up on
                        # the final scores generations at the rc boundary.
                        want = min(len(extra), 2 * slot) if lead else \
                            (len(extra) * slot) // (ST + 1)
                        while ei < want:
                            extra[ei]()
                            ei += 1
                    _emit_av(p, xt_ps, *pend)
                    while ei < len(extra):
                        extra[ei]()
                        ei += 1
                    _normalize_front(p, rc, xt_ps)

            # ---------- phase 0: minimal wire to first score matmul --------
            # Serial-DMA need order: wq, xq0 -> Q chain rc0; wk, xk0 ->
            # K chain st0-3; mask rc0; then stream the rest just-in-time.
            # HAM warmup: dummy matmuls on a memset tile while the first
            # chunks are still on the wire, so the real chains start at the
            # warm (2.4 GHz) rate instead of cold 1.2 GHz.
            warm = const.tile([P, RC], BF16)
            nc.vector.memset(warm, 1.0)
            warm_ps = pjp.tile([P, RC], F32, tag="pj", name="warm_ps")
            for i in range(16):
                nc.tensor.matmul(warm_ps, warm[:, 0:P], warm,
                                 start=(i == 0), stop=(i == 15))

            alloc_pair_tiles(0)
            q0 = QKProj(0, "q")
            k0 = QKProj(0, "k")
            # startup-critical wire: only PAIR 0's weight columns gate the
            # first chains (0.25 MB each instead of the full 1 MB); the
            # remaining columns stream after the phase-0 block — they are
            # first needed by pair-1 projections deep inside pair 0's
            # attention window.
            nc.sync.dma_start(out=wq_sb[:, :, 0:P], in_=wq_v[:, :, 0:P])
            q0.load(0, split=True)
            bias_dmas()
            nc.sync.dma_start(out=wk_sb[:, :, 0:P], in_=wk_v[:, :, 0:P])
            k0.load(0, split=True)
            q0.chain(0)
            k0.chain(0)
            nc.sync.dma_start(out=wv_sb, in_=wv_v)
            xv_load(0)
            mask_chunk(0, half=0)
            k0.load(1)
            xv_load(1)
            xv_load(2)
            mask_chunk(0, half=1)

            # pair 0 rc0: attention starts on K st0-3 only; K s-tiles 4..15
            # and ALL V units stream in lead-paced (2/slot), ordered to
            # match DMA arrival — the in-order PE queue must never block on
            # a not-yet-arrived chunk ahead of ready work.  Every v_unit(st)
            # stays ahead of its AV(st) consumer.
            attn_pair(0, [
                [lambda: v_unit(0), lambda: v_unit(1),
                 lambda: k0.chain(1), lambda: v_unit(2),
                 lambda: k0.load(2), lambda: v_unit(3),
                 lambda: k0.chain(2), lambda: v_unit(4),
                 lambda: v_unit(5),
                 lambda: k0.load(3), lambda: k0.chain(3),
                 lambda: v_unit(6), lambda: v_unit(7),
                 lambda: q0.load(1), lambda: v_unit(8),
                 lambda: v_unit(9), lambda: q0.chain(1),
                 lambda: v_unit(10), lambda: mask_chunk(1),
                 lambda: v_unit(11), lambda: v_unit(12),
                 lambda: v_unit(13), lambda: v_unit(14),
                 lambda: v_unit(15)],
                [lambda: nc.sync.dma_start(out=wq_sb[:, :, P:],
                                           in_=wq_v[:, :, P:]),
                 lambda: q0.load(2), lambda: q0.chain(2),
                 lambda: mask_chunk(2),
                 lambda: nc.sync.dma_start(out=wk_sb[:, :, P:],
                                           in_=wk_v[:, :, P:])],
                [lambda: q0.load(3), lambda: q0.chain(3),
                 lambda: mask_chunk(3), lambda: alloc_pair_tiles(1)] +
                QKProj(1, "q").units(),
                QKProj(1, "k").units(),
            ], lead_rc0=True)
            q2u = QKProj(2, "q").units()
            k2u = QKProj(2, "k").units()
            attn_pair(1, [
                [lambda: alloc_pair_tiles(2)] + q2u[:2],
                q2u[2:],
                k2u,
                [lambda: nc.gpsimd.dma_start(out=wo_sb, in_=wo_v)],
            ])
            q3u = QKProj(3, "q").units()
            k3u = QKProj(3, "k").units()
            attn_pair(2, [
                [lambda: alloc_pair_tiles(3)] + q3u[:2],
                q3u[2:],
                k3u[:2],
                [k3u[2]],
            ])
            # last pair: its own trailing K s-tiles stream into rc0
            # (lead-paced: chain(2) must beat scores st8); O projection
            # r-chunk rc interleaves during rc+1.
            o_units = {rc: [lambda mt_r=mt_r, nn=nn: o_unit(mt_r, nn)
                            for mt_r in range(rc * 4, rc * 4 + 4)
                            for nn in range(D // RC)]
                       for rc in range(NRC)}
            attn_pair(3, [
                k3u[3:],
                o_units[0],
                o_units[1],
                o_units[2],
            ], lead_rc0=True)
            # final r-chunk, software-pipelined: each group of 4 chains
            # (filling all 4 spare psum gens — pj x2 + dead scores x2)
            # issues its kt0-2 accumulations first; those only read pairs
            # 0-2's Xt, so the first group runs UNDER the last normalize
            # flush's DVE/GPSIMD latency.  kt=3 + evict follow the flush.
            def o_prefix(i, mt_r, nn):
                if i % 2 == 1:
                    ps = scp.tile([P, 2 * RC], F32, tag="sc",
                                  name="o_ps_alt")[:, 0:RC]
                else:
                    ps = pjp.tile([P, RC], F32, tag="pj", name="o_ps")
                for kt in range(OKT - 1):
                    nc.tensor.matmul(
                        ps, xt_sb[:, kt, mt_r * P:(mt_r + 1) * P],
                        wo_sb[:, kt, nn * RC:(nn + 1) * RC],
                        start=(kt == 0), stop=False)
                return ps

            def o_suffix(ps, mt_r, nn, i=0):
                kt = OKT - 1
                nc.tensor.matmul(
                    ps, xt_sb[:, kt, mt_r * P:(mt_r + 1) * P],
                    wo_sb[:, kt, nn * RC:(nn + 1) * RC],
                    start=False, stop=True)
                ev = oevp.tile([P, RC], BF16, tag="oev", name="o_ev")
                # the evict drain is the endgame: split copies across the
                # (idle) ScalarE and DVE so the per-op semaphore latencies
                # overlap instead of serializing on one queue.
                if i % 2 == 1:
                    nc.scalar.copy(out=ev, in_=ps)
                else:
                    nc.vector.tensor_copy(out=ev, in_=ps)
                nc.sync.dma_start(
                    out=out[mt_r * P:(mt_r + 1) * P, nn * RC:(nn + 1) * RC],
                    in_=ev)

            finals = [(mt_r, nn) for mt_r in range(12, 16)
                      for nn in range(D // RC)]
            for g in range(0, 8, 4):
                pss = [o_prefix(i, mt_r, nn)
                       for i, (mt_r, nn) in enumerate(finals[g:g + 4])]
                if g == 0:
                    flush_norm()
                for i, (ps, (mt_r, nn)) in enumerate(
                        zip(pss, finals[g:g + 4])):
                    o_suffix(ps, mt_r, nn, i=i)
    nc.finalize()
    return nc


_NC_CACHE = {}


def _get_nc():
    if "nc" not in _NC_CACHE:
        _NC_CACHE["nc"] = build_nc()
    return _NC_CACHE["nc"]


def make_in_maps(query, key, value, mask, Wq, bq, Wk, bk, Wv, bv, Wo, bo):
    from ml_dtypes import bfloat16 as bf16

    query = np.asarray(query, np.float32)
    key = np.asarray(key, np.float32)
    value = np.asarray(value, np.float32)
    mask = np.asarray(mask)

    per_batch = []
    for b in range(B):
        per_batch.append({
            "xqT": np.ascontiguousarray(query[b].T).astype(bf16),
            "xkT": np.ascontiguousarray(key[b].T).astype(bf16),
            "xvT": np.ascontiguousarray(value[b].T).astype(bf16),
            "mskT": np.ascontiguousarray(mask[b].T).astype(bf16),
        })
    per_half = []
    for half in range(2):
        hs = half * DL
        Wq_, Wk_, Wv_, Wo_ = (np.asarray(w, np.float32)
                              for w in (Wq, Wk, Wv, Wo))
        per_half.append({
            "wqT": np.ascontiguousarray(Wq_[hs:hs + DL, :].T).astype(bf16),
            "wkT": np.ascontiguousarray(Wk_[hs:hs + DL, :].T).astype(bf16),
            "wvT": np.ascontiguousarray(Wv_[hs:hs + DL, :].T).astype(bf16),
            "woT": np.ascontiguousarray(Wo_[:, hs:hs + DL].T).astype(bf16),
            "bq": np.ascontiguousarray(np.asarray(bq, np.float32)[hs:hs + DL]),
            "bk": np.ascontiguousarray(np.asarray(bk, np.float32)[hs:hs + DL]),
            "bv": np.ascontiguousarray(np.asarray(bv, np.float32)[hs:hs + DL]),
        })
    in_maps = []
    for c in range(NCORES):
        b, half = c // 2, c % 2
        in_maps.append({**per_batch[b], **per_half[half]})
    return in_maps


def assemble(results, bo):
    """results: per-core dicts with 'out' partials; sum head-halves + bias."""
    bo = np.asarray(bo, np.float32)
    full = np.empty((B, S, D), dtype=np.float32)
    for b in range(B):
        full[b] = (np.asarray(results[2 * b]["out"], np.float32) +
                   np.asarray(results[2 * b + 1]["out"], np.float32) + bo)
    return full


def kernel(query, key, value, mask, Wq, bq, Wk, bk, Wv, bv, Wo, bo):
    from concourse.bass_utils import run_bass_kernel_spmd

    nc = _get_nc()
    in_maps = make_in_maps(query, key, value, mask,
                           Wq, bq, Wk, bk, Wv, bv, Wo, bo)
    res = run_bass_kernel_spmd(nc, in_maps, list(range(NCORES)))
    return assemble(res.results, bo)
